# revision 1
# baseline (speedup 1.0000x reference)
"""CTRGC kernel for Trainium2 (Bass/Tile), 8-core SPMD.

Sharding: core k handles branch b=k//2 (of jo,bo,jm,bm) and batch half
h=k%2 (16 of 32 samples). Pure data parallelism; per-core weights differ
via in_maps only (same program on all cores).

Per (branch, sample) math (C=64, R=8, T=256, V=25):
  xm  = mean_t x                         [64,25]
  x1  = W1 xm + b1 ; x2 = W2 xm + b2     [8,25]
  att[r,u,v] = tanh(x1[r,u]-x2[r,v])     [8,25,25]
  a   = W4 att + b4 + A                  [64,25,25]
  x3  = W3 x + b3                        [64,256,25]
  out[c,t,u] = sum_v a[c,u,v] x3[c,t,v]  [64,256,25]

Device layout choices (host may freely pre/post-transpose: it's part of
shard/unshard):
  x arrives v-major [65,25,256] with partition 64 = ones (exact b3 via
  widened W3 lhsT). x3 produced [64,(v,t)] via 13 N=512 matmuls (one PSUM
  bank each, since 512 = 2 v-rows x 256 t). t-sum via 16 PSUM-accumulated
  identity matmuls + DVE innermost reduce. att via broadcast-sub (DVE) +
  tanh (ACT, bias=b1-b2). a via W4 matmul + host-precomputed (A+b4) add.
  SBUF->SBUF DMA bridges (1KB runs) transpose x3/a to v-on-partitions in
  4 row-groups (i = c//16), then step-6 runs as 16-way tile_position-packed
  25x25xT matmuls (tile (i,j) = channel 16i+4j+r in round r). Output stays
  in device layout [(j|u), r, i, t]; host unshards.
"""

import numpy as np

try:
    import concourse  # noqa: F401
except ImportError:  # pragma: no cover
    import sys
    sys.path.insert(0, "/opt/trn_rl_repo")

_CACHE = {}


def _build_nc():
    STAGE = 5  # full pipeline (lower values were used for HW bisection)
    from concourse import bacc, tile
    from concourse.bass import mybir

    f32 = mybir.dt.float32
    ALU = mybir.AluOpType
    ACT = mybir.ActivationFunctionType

    nc = bacc.Bacc(None, target_bir_lowering=False)
    x_d = nc.declare_dram_parameter("x", [16, 65, 25, 256], f32, isOutput=False)
    w3t_d = nc.declare_dram_parameter("w3t", [65, 64], f32, isOutput=False)
    w12t_d = nc.declare_dram_parameter("w12t", [64, 16], f32, isOutput=False)
    w4t_d = nc.declare_dram_parameter("w4t", [8, 64], f32, isOutput=False)
    b12_d = nc.declare_dram_parameter("b12", [8, 1], f32, isOutput=False)
    apb4_d = nc.declare_dram_parameter("apb4", [64, 25, 25], f32, isOutput=False)
    eye_d = nc.declare_dram_parameter("eye64", [64, 64], f32, isOutput=False)
    out_d = nc.declare_dram_parameter("out", [16, 128, 4, 4, 256], f32, isOutput=True)
    # DRAM scratch for the c->v partition transposes (store strided, load dense)
    x3scr = [nc.dram_tensor(f"x3scr{p}", [100, 16, 256], f32) for p in range(2)]
    ascr = [nc.dram_tensor(f"ascr{p}", [100, 16, 25], f32) for p in range(2)]

    with tile.TileContext(nc) as tc:
        with (
            tc.tile_pool(name="const", bufs=1) as cpool,
            tc.tile_pool(name="xin", bufs=2) as xpool,
            tc.tile_pool(name="x3", bufs=2) as x3pool,
            tc.tile_pool(name="x3t", bufs=2) as x3tpool,
            tc.tile_pool(name="outp", bufs=2) as outpool,
            tc.tile_pool(name="small", bufs=2) as spool,
            tc.tile_pool(name="ps_small", bufs=2, space="PSUM") as ps_s,
            tc.tile_pool(name="ps_x3", bufs=2, space="PSUM") as ps_x3,
            tc.tile_pool(name="ps_out", bufs=2, space="PSUM") as ps_o,
        ):
            w3t = cpool.tile([65, 64], f32)
            nc.sync.dma_start(w3t[:], w3t_d[:])
            w12t = cpool.tile([64, 16], f32)
            nc.sync.dma_start(w12t[:], w12t_d[:])
            w4t = cpool.tile([8, 64], f32)
            nc.sync.dma_start(w4t[:], w4t_d[:])
            b12 = cpool.tile([8, 1], f32)
            nc.sync.dma_start(b12[:], b12_d[:])
            apb4 = cpool.tile([64, 25, 25], f32)
            nc.sync.dma_start(apb4[:], apb4_d[:])
            eye = cpool.tile([64, 64], f32)
            nc.sync.dma_start(eye[:], eye_d[:])
            apb4_f = apb4[:].rearrange("c v u -> c (v u)")

            for n in range(16):
                xt = xpool.tile([65, 25, 256], f32, tag="xt")
                nc.sync.dma_start(xt[:], x_d[n])

                # --- t-sum of x via accumulated identity matmuls ---
                if STAGE < 3:
                    a_sb = spool.tile([64, 25, 25], f32, tag="a_sb")
                    nc.gpsimd.memset(a_sb[:], 0.0)
                xsum_ps = None
                if STAGE >= 3:
                    xsum_ps = ps_s.tile([64, 25, 16], f32, tag="small")
                if STAGE >= 3:
                    for s in range(16):
                        nc.tensor.matmul(
                            xsum_ps[:],
                            eye[:],
                            xt[0:64, :, 16 * s : 16 * (s + 1)],
                            start=(s == 0),
                            stop=(s == 15),
                        )
                    xsum_sb = spool.tile([64, 25], f32, tag="xsum_sb")
                    nc.vector.tensor_reduce(
                        out=xsum_sb[:], in_=xsum_ps[:], axis=mybir.AxisListType.X,
                        op=ALU.add,
                    )

                    # --- x1/x2 (weights pre-scaled by 1/T on host) ---
                    x12_ps = ps_s.tile([8, 2, 25], f32, tag="small")
                    nc.tensor.matmul(x12_ps[:, 0, :], w12t[:, 0:8], xsum_sb[:],
                                     start=True, stop=True)
                    nc.tensor.matmul(x12_ps[:, 1, :], w12t[:, 8:16], xsum_sb[:],
                                     start=True, stop=True)
                    x12_sb = spool.tile([8, 2, 25], f32, tag="x12_sb")
                    nc.vector.tensor_copy(x12_sb[:], x12_ps[:])

                    # --- att[r,v,u] = tanh(x1[r,u] - x2[r,v] + (b1-b2)[r]) ---
                    attp = spool.tile([8, 25, 25], f32, tag="attp")
                    x1b = x12_sb[:, 0:1, :].broadcast_to([8, 25, 25])
                    x2b = x12_sb[:, 1:2, :].rearrange("r o v -> r v o").broadcast_to(
                        [8, 25, 25])
                    nc.vector.tensor_tensor(attp[:], x1b, x2b, op=ALU.subtract)
                    att = spool.tile([8, 25, 25], f32, tag="att")
                    nc.scalar.activation(att[:], attp[:], ACT.Tanh, bias=b12[:],
                                         scale=1.0)
                    att_f = att[:].rearrange("r v u -> r (v u)")

                    # --- a[c,(v,u)] = W4 att + (A + b4) ---
                    a_sb = spool.tile([64, 25, 25], f32, tag="a_sb")
                    a_f = a_sb[:].rearrange("c v u -> c (v u)")
                    for (o0, o1) in ((0, 512), (512, 625)):
                        a_ps = ps_s.tile([64, o1 - o0], f32, tag="small")
                        nc.tensor.matmul(a_ps[:], w4t[:], att_f[:, o0:o1],
                                         start=True, stop=True)
                        nc.vector.tensor_tensor(a_f[:, o0:o1], a_ps[:],
                                                apb4_f[:, o0:o1], op=ALU.add)

                # --- x3 = W3 x + b3, v-major, 13 one-bank chunks ---
                x3sb = x3pool.tile([64, 25, 256], f32, tag="x3sb")
                if STAGE < 2:
                    nc.gpsimd.memset(x3sb[:], 0.0)
                if STAGE >= 2:
                    for m in range(13):
                        v0 = 2 * m
                        nv = 2 if m < 12 else 1
                        x3_ps = ps_x3.tile([64, 2, 256], f32, tag="x3ps")
                        nc.tensor.matmul(x3_ps[:, 0:nv, :], w3t[:],
                                         xt[:, v0 : v0 + nv, :],
                                         start=True, stop=True)
                        if m % 2 == 0:
                            nc.vector.tensor_copy(x3sb[:, v0 : v0 + nv, :],
                                                  x3_ps[:, 0:nv, :])
                        else:
                            nc.scalar.activation(x3sb[:, v0 : v0 + nv, :],
                                                 x3_ps[:, 0:nv, :], ACT.Copy)

                # --- bridges: v onto partitions via DRAM bounce ---
                # store side does the (c,v)->(v,c) transpose with 1KB runs;
                # load side is dense. Ping-pong scratch keeps nb's pipelined.
                aT = None; x3T = None
                if STAGE >= 4:
                    asc = ascr[n % 2]
                    xsc = x3scr[n % 2]
                    aT = spool.tile([128, 16, 32], f32, tag="aT")
                    nc.gpsimd.memset(aT[:], 0.0)
                    x3T = x3tpool.tile([128, 16, 256], f32, tag="x3T")
                    for i in range(4):
                        nc.gpsimd.dma_start(
                            asc[25 * i : 25 * (i + 1)].rearrange("v c u -> c v u"),
                            a_sb[16 * i : 16 * (i + 1), :, :],
                        )
                        nc.gpsimd.dma_start(
                            xsc[25 * i : 25 * (i + 1)].rearrange("v c t -> c v t"),
                            x3sb[16 * i : 16 * (i + 1), :, :],
                        )
                    for i in range(4):
                        nc.gpsimd.dma_start(aT[32 * i : 32 * i + 25, :, 0:25],
                                            asc[25 * i : 25 * (i + 1)])
                        nc.gpsimd.dma_start(x3T[32 * i : 32 * i + 25, :, :],
                                            xsc[25 * i : 25 * (i + 1)])

                # --- step 6: out[c,t,u] via 16-way packed 25x25 matmuls ---
                outsb = outpool.tile([128, 4, 4, 256], f32, tag="outsb")
                if STAGE < 5:
                    nc.gpsimd.memset(outsb[:], 0.0)
                if STAGE >= 5:
                    for r in range(4):
                        for ph in range(2):
                            ops = ps_o.tile([128, 2, 512], f32, tag="ops")
                            for di in range(2):
                                i = 2 * ph + di
                                for j in range(4):
                                    cc = 4 * j + r
                                    nc.tensor.matmul(
                                        ops[32 * j : 32 * j + 32, di, 0:256],
                                        aT[32 * i : 32 * i + 25, cc, :],
                                        x3T[32 * i : 32 * i + 25, cc, :],
                                        start=True, stop=True,
                                        tile_position=(32 * i, 32 * j),
                                    )
                            if (2 * r + ph) % 2 == 0:
                                nc.vector.tensor_copy(
                                    outsb[:, r, 2 * ph : 2 * ph + 2, :],
                                    ops[:, :, 0:256])
                            else:
                                nc.scalar.activation(
                                    outsb[:, r, 2 * ph : 2 * ph + 2, :],
                                    ops[:, :, 0:256], ACT.Copy)
                nc.sync.dma_start(out_d[n], outsb[:])

    nc.compile()
    return nc


def _prep_core(x_half, A_b, W1, B1, W2, B2, W3, B3, W4, B4):
    f = np.float32
    n = x_half.shape[0]
    xv = np.empty((n, 65, 25, 256), dtype=f)
    xv[:, :64] = x_half.transpose(0, 1, 3, 2)
    xv[:, 64] = 1.0
    w3t = np.empty((65, 64), dtype=f)
    w3t[:64] = W3.T
    w3t[64] = B3
    w12t = np.concatenate([(W1 / 256.0).T, (W2 / 256.0).T], axis=1).astype(f)
    w4t = np.ascontiguousarray(W4.T).astype(f)
    b12 = (B1 - B2).reshape(8, 1).astype(f)
    apb4 = (A_b.T[None, :, :] + B4[:, None, None]).astype(f)
    return {
        "x": np.ascontiguousarray(xv),
        "w3t": np.ascontiguousarray(w3t),
        "w12t": np.ascontiguousarray(w12t),
        "w4t": w4t,
        "b12": b12,
        "apb4": np.ascontiguousarray(apb4),
        "eye64": np.eye(64, dtype=f),
    }


def kernel(**inputs):
    from concourse.bass_utils import run_bass_kernel_spmd

    if "nc" not in _CACHE:
        _CACHE["nc"] = _build_nc()
    nc = _CACHE["nc"]

    A = np.asarray(inputs["A"], dtype=np.float32)
    xs = [np.asarray(inputs[k], dtype=np.float32) for k in ("jo", "bo", "jm", "bm")]
    W = {k: np.asarray(inputs[k], dtype=np.float32)
         for k in ("W1", "B1", "W2", "B2", "W3", "B3", "W4", "B4")}

    in_maps = []
    for k in range(8):
        b, h = k // 2, k % 2
        in_maps.append(_prep_core(
            xs[b][16 * h : 16 * (h + 1)], A[b],
            W["W1"][b], W["B1"][b], W["W2"][b], W["B2"][b],
            W["W3"][b], W["B3"][b], W["W4"][b], W["B4"][b],
        ))

    res = run_bass_kernel_spmd(nc, in_maps, list(range(8))).results

    outs = []
    for b in range(4):
        parts = []
        for h in range(2):
            o = np.asarray(res[2 * b + h]["out"]).reshape(16, 4, 32, 4, 4, 256)
            o = o[:, :, :25]  # [n', j, u, r, i, t]
            o = o.transpose(0, 4, 1, 3, 5, 2)  # [n', i, j, r, t, u]
            parts.append(o.reshape(16, 64, 256, 25))
        outs.append(np.concatenate(parts, axis=0))
    return tuple(outs)



# revision 2
# speedup vs baseline: 186.1913x; 186.1913x over previous
"""CTRGC kernel for Trainium2 (Bass/Tile), 8-core SPMD, bf16.

Sharding: core k = branch k//2 (of jo,bo,jm,bm) x batch half k%2 (16 of 32
samples). Within a core, samples are processed in PAIRS on 128 partitions:
partition 64*s + c = channel c of pair-sample s; all matmuls run as two
diagonal tile_position quadrants (A: rows/cols 0-63, B: rows/cols 64-127),
so the two samples never mix in the contraction.

Per (branch, sample) math (C=64, R=8, T=256, V=25):
  xm  = mean_t x; x1 = W1 xm + b1; x2 = W2 xm + b2      [8,25]
  att[r,u,v] = tanh(x1[r,u]-x2[r,v])                     [8,25,25]
  a   = W4 att + b4 + A                                  [64,25,25]
  x3  = W3 x + b3                                        [64,25v,256t]
  out[c,t,u] = sum_v a[c,u,v] x3[c,t,v]                  [64,256,25]

Everything is bf16 (inputs host-cast; PSUM accumulation f32; rel-err ~4e-3
vs the 2e-2 gate). t-sum via PSUM-accumulated identity matmuls (per-sample
banks); b3 added during PSUM evacuation (DVE tensor_tensor / ACT
Identity+bias); b1-b2 via tanh bias; A+b4 host-folded into apb4 (DVE add).
x3/a are transposed to v-on-partitions via a DRAM bounce (dense bf16 store
+ 2 strided loads each) into persistent double-buffered tiles with sample
s on partition band 64s. Step 6 runs as 4-way column-packed K=25 matmuls
(tile_position=(64s, 32j), M=32 via zero-padded aT), 256 per pair-of-
samples. The per-pair program is software-pipelined: step6+store of pair
p-1 are emitted after the front-end of pair p so no engine queue stalls on
the transpose bounce. loop_reps wraps the whole thing in a hardware For_i
loop (used only for timing).
"""

import numpy as np

try:
    import concourse  # noqa: F401
except ImportError:  # pragma: no cover
    import sys
    sys.path.insert(0, "/opt/trn_rl_repo")

_CACHE = {}


def _build_nc(loop_reps=1):
    from concourse import bacc, tile
    from concourse.bass import mybir

    f32 = mybir.dt.float32
    bf16 = mybir.dt.bfloat16
    ALU = mybir.AluOpType
    ACT = mybir.ActivationFunctionType
    AX = mybir.AxisListType

    nc = bacc.Bacc(None, target_bir_lowering=False)
    x_d = nc.declare_dram_parameter("x", [8, 128, 25, 256], bf16, isOutput=False)
    w3t_d = nc.declare_dram_parameter("w3t", [128, 64], bf16, isOutput=False)
    w12t_d = nc.declare_dram_parameter("w12t", [128, 2, 8], bf16, isOutput=False)
    w4t_d = nc.declare_dram_parameter("w4t", [128, 64], bf16, isOutput=False)
    eye_d = nc.declare_dram_parameter("eye", [128, 64], bf16, isOutput=False)
    b12_d = nc.declare_dram_parameter("b12", [128, 1], f32, isOutput=False)
    b3_d = nc.declare_dram_parameter("b3", [128, 1], f32, isOutput=False)
    apb4_d = nc.declare_dram_parameter("apb4", [128, 25, 25], bf16, isOutput=False)
    out_d = nc.declare_dram_parameter("out", [8, 128, 2, 16, 256], bf16,
                                      isOutput=True)
    x3scr = [nc.dram_tensor(f"x3scr{k}", [128, 25, 256], bf16)
             for k in range(2)]
    ascr = [nc.dram_tensor(f"ascr{k}", [128, 25, 25], bf16)
            for k in range(2)]

    with tile.TileContext(nc) as tc:
        with (
            tc.tile_pool(name="const", bufs=1) as cpool,
            tc.tile_pool(name="xin", bufs=2) as xpool,
            tc.tile_pool(name="x3", bufs=2) as x3pool,
            tc.tile_pool(name="outp", bufs=2) as outpool,
            tc.tile_pool(name="small", bufs=2) as spool,
            tc.tile_pool(name="ps_ts", bufs=1, space="PSUM") as ps_ts,
            tc.tile_pool(name="ps_x12", bufs=1, space="PSUM") as ps_x12,
            tc.tile_pool(name="ps_a", bufs=1, space="PSUM") as ps_a,
            tc.tile_pool(name="ps_x3", bufs=2, space="PSUM") as ps_x3,
            tc.tile_pool(name="ps_s6", bufs=2, space="PSUM") as ps_s6,
        ):
            w3t = cpool.tile([128, 64], bf16)
            nc.sync.dma_start(w3t[:], w3t_d[:])
            w12t = cpool.tile([128, 2, 8], bf16)
            nc.sync.dma_start(w12t[:], w12t_d[:])
            w4t = cpool.tile([128, 64], bf16)
            nc.sync.dma_start(w4t[:], w4t_d[:])
            eye = cpool.tile([128, 64], bf16)
            nc.sync.dma_start(eye[:], eye_d[:])
            b12 = cpool.tile([128, 1], f32)
            nc.sync.dma_start(b12[:], b12_d[:])
            b3 = cpool.tile([128, 1], f32)
            nc.sync.dma_start(b3[:], b3_d[:])
            apb4 = cpool.tile([128, 25, 25], bf16)
            nc.sync.dma_start(apb4[:], apb4_d[:])
            apb4_f = apb4[:].rearrange("p v u -> p (v u)")

            # Persistent double-buffered transpose targets. Off-diagonal
            # zeros of aT and pad rows 125-127 of x3T are written once.
            aTs = [cpool.tile([128, 64, 32], bf16, tag=f"aT{k}",
                              name=f"aT{k}") for k in range(2)]
            x3Ts = [cpool.tile([128, 64, 256], bf16, tag=f"x3T{k}",
                               name=f"x3T{k}") for k in range(2)]
            for k in range(2):
                nc.gpsimd.memset(aTs[k][:], 0.0)

            def pair_body(p):
                aT = aTs[p % 2]
                x3T = x3Ts[p % 2]
                xt = xpool.tile([128, 25, 256], bf16, tag="xt")
                nc.sync.dma_start(xt[:], x_d[p])

                # --- t-sum via accumulated identity matmuls ---
                ts_ps = ps_ts.tile([128, 2, 512], f32, tag="ts")
                tsA = ts_ps[0:64, 0, 0:400].rearrange("p (v k) -> p v k", v=25)
                tsB = ts_ps[64:128, 1, 0:400].rearrange(
                    "p (v k) -> p v k", v=25)
                for k in range(16):
                    nc.tensor.matmul(tsA, eye[0:64],
                                     xt[0:64, :, 16 * k:16 * k + 16],
                                     start=(k == 0), stop=(k == 15),
                                     tile_position=(0, 0))
                for k in range(16):
                    nc.tensor.matmul(tsB, eye[64:128],
                                     xt[64:128, :, 16 * k:16 * k + 16],
                                     start=(k == 0), stop=(k == 15),
                                     tile_position=(64, 64))
                xsum = spool.tile([128, 25], bf16, tag="xsum")
                with nc.allow_low_precision(
                        reason="16-partial f32 sum stored bf16 for matmul"):
                    nc.vector.tensor_reduce(out=xsum[0:64], in_=tsA,
                                            axis=AX.X, op=ALU.add)
                    nc.vector.tensor_reduce(out=xsum[64:128], in_=tsB,
                                            axis=AX.X, op=ALU.add)

                # --- x1/x2 (weights pre-scaled by 1/T on host) ---
                x12_ps = ps_x12.tile([128, 2, 25], f32, tag="x12")
                nc.tensor.matmul(x12_ps[0:8, 0, :], w12t[0:64, 0, :],
                                 xsum[0:64, :], start=True, stop=True,
                                 tile_position=(0, 0))
                nc.tensor.matmul(x12_ps[0:8, 1, :], w12t[0:64, 1, :],
                                 xsum[0:64, :], start=True, stop=True,
                                 tile_position=(0, 0))
                nc.tensor.matmul(x12_ps[32:40, 0, :], w12t[64:128, 0, :],
                                 xsum[64:128, :], start=True, stop=True,
                                 tile_position=(64, 32))
                nc.tensor.matmul(x12_ps[32:40, 1, :], w12t[64:128, 1, :],
                                 xsum[64:128, :], start=True, stop=True,
                                 tile_position=(64, 32))
                x12_sb = spool.tile([128, 2, 25], f32, tag="x12sb")
                nc.gpsimd.memset(x12_sb[:], 0.0)
                nc.vector.tensor_copy(x12_sb[0:8], x12_ps[0:8])
                nc.vector.tensor_copy(x12_sb[32:40], x12_ps[32:40])

                # --- att[r,v,u] = tanh(x1[r,u] - x2[r,v] + (b1-b2)[r]) ---
                attp = spool.tile([128, 25, 25], bf16, tag="attp")
                x1b = x12_sb[0:40, 0:1, :].broadcast_to([40, 25, 25])
                x2b = x12_sb[0:40, 1:2, :].rearrange(
                    "r o v -> r v o").broadcast_to([40, 25, 25])
                nc.gpsimd.tensor_tensor(attp[0:40], x1b, x2b,
                                        op=ALU.subtract)
                att = spool.tile([128, 25, 25], bf16, tag="att")
                nc.scalar.activation(att[0:40], attp[0:40], ACT.Tanh,
                                     bias=b12[0:40], scale=1.0)
                att_f = att[:].rearrange("p v u -> p (v u)")

                # --- a[c,(v,u)] = W4 att + (A + b4) ---
                a_sb = spool.tile([128, 25, 25], bf16, tag="a_sb")
                a_f = a_sb[:].rearrange("p v u -> p (v u)")
                for (o0, o1) in ((0, 512), (512, 625)):
                    a_ps = ps_a.tile([128, 512], f32, tag="a")
                    nc.tensor.matmul(a_ps[0:64, 0:o1 - o0], w4t[0:8, :],
                                     att_f[0:8, o0:o1], start=True,
                                     stop=True, tile_position=(0, 0))
                    nc.tensor.matmul(a_ps[64:128, 0:o1 - o0],
                                     w4t[32:40, :], att_f[32:40, o0:o1],
                                     start=True, stop=True,
                                     tile_position=(32, 64))
                    nc.vector.tensor_tensor(a_f[:, o0:o1],
                                            a_ps[:, 0:o1 - o0],
                                            apb4_f[:, o0:o1], op=ALU.add)

                # --- aT: transpose via DRAM bounce, sample s on band 64s ---
                asc = ascr[p % 2]
                nc.sync.dma_start(asc[:], a_sb[:])
                for s in (0, 1):
                    nc.scalar.dma_start(
                        aT[64 * s:64 * s + 25, :, 0:25],
                        asc[64 * s:64 * s + 64].rearrange("c v u -> v c u"))

                # --- x3 = W3 x + b3 ---
                x3sb = x3pool.tile([128, 25, 256], bf16, tag="x3sb")
                for m in range(13):
                    v0 = 2 * m
                    nv = 2 if m < 12 else 1
                    x3_ps = ps_x3.tile([128, 2, 256], f32, tag="x3")
                    nc.tensor.matmul(x3_ps[0:64, 0:nv, :], w3t[0:64],
                                     xt[0:64, v0:v0 + nv, :], start=True,
                                     stop=True, tile_position=(0, 0))
                    nc.tensor.matmul(x3_ps[64:128, 0:nv, :], w3t[64:128],
                                     xt[64:128, v0:v0 + nv, :], start=True,
                                     stop=True, tile_position=(64, 64))
                    dst = x3sb[:, v0:v0 + nv, :].rearrange("p v t -> p (v t)")
                    srcf = x3_ps[:, 0:nv, :].rearrange("p v t -> p (v t)")
                    bb = b3[:, 0:1].broadcast_to([128, nv * 256])
                    if m % 5 < 3:
                        nc.vector.tensor_tensor(dst, srcf, bb, op=ALU.add)
                    else:
                        nc.scalar.activation(dst, srcf, ACT.Identity,
                                             bias=b3[:], scale=1.0)

                # --- x3T: transpose via DRAM bounce, sample s on band 64s ---
                xsc = x3scr[p % 2]
                nc.sync.dma_start(xsc[:], x3sb[:])
                for s in (0, 1):
                    nc.scalar.dma_start(
                        x3T[64 * s:64 * s + 25, :, :],
                        xsc[64 * s:64 * s + 64].rearrange("c v t -> v c t"))

            def back_body(p):
                aT = aTs[p % 2]
                x3T = x3Ts[p % 2]
                # --- step 6: 4-way col-packed K=25 rounds ---
                out_sb = outpool.tile([128, 2, 16, 256], bf16, tag="out_sb")
                for ss in (0, 1):
                    for q in range(16):
                        s6 = ps_s6.tile([128, 256], f32, tag="s6")
                        for j in range(4):
                            c = 4 * q + j
                            nc.tensor.matmul(
                                s6[32 * j:32 * j + 32, :],
                                aT[64 * ss:64 * ss + 25, c, :],
                                x3T[64 * ss:64 * ss + 25, c, :],
                                start=True, stop=True,
                                tile_position=(64 * ss, 32 * j))
                        dst = out_sb[:, ss, q, :]
                        if (16 * ss + q) % 5 < 3:
                            nc.vector.tensor_copy(dst, s6[:])
                        else:
                            nc.scalar.activation(dst, s6[:], ACT.Copy)
                nc.sync.dma_start(out_d[p], out_sb[:])

            if loop_reps > 1:
                _hints = (mybir.EngineType.PE, mybir.EngineType.DVE,
                          mybir.EngineType.Activation, mybir.EngineType.SP,
                          mybir.EngineType.Pool)
                with tc.For_i(0, loop_reps, 1, hint_engines=_hints):
                    for p in range(8):
                        pair_body(p)
                        if p > 0:
                            back_body(p - 1)
                    back_body(7)
            else:
                for p in range(8):
                    pair_body(p)
                    if p > 0:
                        back_body(p - 1)
                back_body(7)

    nc.compile()
    return nc


def _prep_core(x_half, A_b, W1, B1, W2, B2, W3, B3, W4, B4):
    f = np.float32
    import ml_dtypes
    bf16 = ml_dtypes.bfloat16

    n = x_half.shape[0]  # 16
    xv = x_half.transpose(0, 1, 3, 2).reshape(8, 2 * 64, 25, 256)
    dup = lambda a: np.concatenate([a, a], axis=0)

    w3t = dup(np.ascontiguousarray(W3.T))                       # [128, 64]
    w12t_h = np.stack([(W1 / 256.0).T, (W2 / 256.0).T], axis=1)  # [64,2,8]
    w12t = dup(w12t_h)
    w4t = np.zeros((128, 64), f)
    w4t[0:8] = W4.T
    w4t[32:40] = W4.T
    eye = dup(np.eye(64, dtype=f))
    b12 = np.zeros((128, 1), f)
    b12[0:8, 0] = B1 - B2
    b12[32:40, 0] = B1 - B2
    b3 = np.zeros((128, 1), f)
    b3[0:64, 0] = B3
    b3[64:128, 0] = B3
    apb4 = dup((A_b.T[None, :, :] + B4[:, None, None]).astype(f))  # [128,25,25]

    return {
        "x": np.ascontiguousarray(xv).astype(bf16),
        "w3t": w3t.astype(bf16),
        "w12t": np.ascontiguousarray(w12t).astype(bf16),
        "w4t": w4t.astype(bf16),
        "eye": eye.astype(bf16),
        "b12": b12,
        "b3": b3,
        "apb4": np.ascontiguousarray(apb4).astype(bf16),
    }


def _unshard_core(outbuf):
    """outbuf [8, 128, 2, 16, 256] bf16 -> [16, 64, 256, 25] f32."""
    ob = np.asarray(outbuf, dtype=np.float32)      # [8,128,2,16,256]
    ob = ob.reshape(8, 4, 32, 2, 16, 256)[:, :, 0:25]  # [p,j,u,s,g,t]
    ob = ob.transpose(0, 3, 4, 1, 5, 2)            # [p,s,g,j,t,u]
    ob = ob.reshape(16, 64, 256, 25)               # c = 4g + j
    return ob


def kernel(**inputs):
    from concourse.bass_utils import run_bass_kernel_spmd

    if "nc" not in _CACHE:
        _CACHE["nc"] = _build_nc()
    nc = _CACHE["nc"]

    A = np.asarray(inputs["A"], dtype=np.float32)
    xs = [np.asarray(inputs[k], dtype=np.float32)
          for k in ("jo", "bo", "jm", "bm")]
    W = {k: np.asarray(inputs[k], dtype=np.float32)
         for k in ("W1", "B1", "W2", "B2", "W3", "B3", "W4", "B4")}

    in_maps = []
    for k in range(8):
        b, h = k // 2, k % 2
        in_maps.append(_prep_core(
            xs[b][16 * h:16 * (h + 1)], A[b],
            W["W1"][b], W["B1"][b], W["W2"][b], W["B2"][b],
            W["W3"][b], W["B3"][b], W["W4"][b], W["B4"][b],
        ))

    res = run_bass_kernel_spmd(nc, in_maps, list(range(8))).results

    outs = []
    for b in range(4):
        parts = [_unshard_core(res[2 * b + h]["out"]) for h in range(2)]
        outs.append(np.concatenate(parts, axis=0))
    return tuple(outs)
